# revision 1
# baseline (speedup 1.0000x reference)
"""DeepFM forward kernel for 8 Trainium2 NeuronCores.

Strategy (data-parallel, per the sharding hint): shard the batch of 2048
rows across 8 cores (256 rows each); replicate the embedding table, FM
linear weight, and MLP params.

On-host prep inside kernel():
  - x (int64 [2048, 8]) -> global row ids int32, packed per-core as
    [128 partitions, 16] (2 batch-tiles x 8 fields per partition).
  - emb_table [60000, 16] and w_lin [60000] are packed into one augmented
    table [60000, 20] (16 emb + 1 w_lin + 3 pad) so ONE indirect-DMA
    gather fetches both the embedding row and its FM-linear weight.
  - all MLP weights/biases (and b_lin) are packed into a single f32 blob
    [128, 582] so they load with ONE dma_start.

On-device per core:
  - 1 blob load, 1 xoff load, 1 indirect gather (2048 descriptors x 80B),
    compute (FM via DVE tensor ops, MLP via PE matmuls in transposed-
    activation form: only one PE transpose needed), 1 output store.
"""

import numpy as np

import concourse.bass as bass
import concourse.bacc as bacc
import concourse.mybir as mybir
import concourse.tile as tile
from concourse.bass_utils import run_bass_kernel_spmd

N_CORES = 8
B = 2048
BC = B // N_CORES  # 256 rows per core
NT = BC // 128     # 2 tiles of 128 rows
F = 8              # fields
D = 16             # embed dim
TABW = 20          # augmented table row: 16 emb + 1 w_lin + 3 pad
FIELD_DIMS = [50000, 5000, 2000, 1000, 1000, 500, 300, 200]
OFFSETS = np.concatenate([[0], np.cumsum(FIELD_DIMS)[:-1]]).astype(np.int64)
INPUT_DIM = int(np.sum(FIELD_DIMS))  # 60000
H1, H2, H3 = 256, 128, 64

# blob column layout (f32, [128, BLOBW])
C_W1 = 0             # w1 [128, 256]
C_W2 = C_W1 + H1     # w2 chunks [128, 128] x2
C_W3 = C_W2 + H2 * 2  # w3 [128, 64]
C_WL = C_W3 + H3     # w_last [64] in partitions 0..63
C_B1 = C_WL + 1      # b1 [256] as 2 cols of 128
C_B2 = C_B1 + 2      # b2 [128]
C_B3 = C_B2 + 1      # b3 [64] in partitions 0..63
C_BLIN = C_B3 + 1    # b_lin broadcast to all partitions
C_ID = C_BLIN + 1    # 128x128 identity (for PE transpose-by-matmul)
BLOBW = C_ID + 128

_CACHE = {}


def build_program():
    """Build the single-core Bass/Tile program (SPMD: same NEFF on all cores)."""
    f32 = mybir.dt.float32
    i32 = mybir.dt.int32
    Alu = mybir.AluOpType
    Act = mybir.ActivationFunctionType

    # Bacc (not raw Bass): its lowering passes split/move multi-sem waits
    # (move_matmul_waits_to_ldweights, generate_event_semaphores) that the
    # TRN2 PE instruction encoding can't hold.
    nc = bacc.Bacc(None, target_bir_lowering=False)
    tab = nc.dram_tensor("tab", [INPUT_DIM, TABW], f32, kind="ExternalInput")
    blob = nc.dram_tensor("blob", [128, BLOBW], f32, kind="ExternalInput")
    xoff = nc.dram_tensor("xoff", [128, NT * F], i32, kind="ExternalInput")
    y = nc.dram_tensor("y", [128, NT], f32, kind="ExternalOutput")

    with tile.TileContext(nc) as tc:
        with (
            tc.tile_pool(name="sb", bufs=2) as sp,
            tc.tile_pool(name="cst", bufs=1) as cp,
            tc.tile_pool(name="ps", bufs=1, space="PSUM") as pp,
        ):
            blob_t = cp.tile([128, BLOBW], f32)
            nc.sync.dma_start(out=blob_t[:], in_=blob[:])
            xoff_t = cp.tile([128, NT * F], i32)
            nc.sync.dma_start(out=xoff_t[:], in_=xoff[:])
            # gather: g[p, j*20:(j+1)*20] = tab[xoff[p, j], :].
            # HW SWDGE ucode supports exactly ONE index per partition per
            # indirect DMA (the simulator's multi-index semantics do not
            # hold on hardware), so issue one call per j.
            g = cp.tile([128, NT * F * TABW], f32)
            for j in range(NT * F):
                nc.gpsimd.indirect_dma_start(
                    out=g[:, j * TABW:(j + 1) * TABW],
                    out_offset=None,
                    in_=tab[:],
                    in_offset=bass.IndirectOffsetOnAxis(ap=xoff_t[:, j:j + 1], axis=0),
                )

            y_sb = cp.tile([128, NT], f32)
            for i in range(NT):
                g3 = g[:, i * F * TABW:(i + 1) * F * TABW].rearrange(
                    "p (f d) -> p f d", f=F
                )
                # contiguous copy of the embedding part: hc[p, f*16+d]
                hc = sp.tile([128, F * D], f32)
                hc3 = hc[:].rearrange("p (f d) -> p f d", f=F)
                nc.vector.tensor_copy(out=hc3, in_=g3[:, :, 0:D])

                # FM: fm2 = sum_d (sum_f h)^2 - sum_{f,d} h^2
                s4 = sp.tile([128, 4 * D], f32)
                nc.vector.tensor_add(
                    out=s4[:].rearrange("p (f d) -> p f d", f=4),
                    in0=hc3[:, 0:4, :], in1=hc3[:, 4:8, :],
                )
                s43 = s4[:].rearrange("p (f d) -> p f d", f=4)
                s2 = sp.tile([128, 2 * D], f32)
                nc.vector.tensor_add(
                    out=s2[:].rearrange("p (f d) -> p f d", f=2),
                    in0=s43[:, 0:2, :], in1=s43[:, 2:4, :],
                )
                s23 = s2[:].rearrange("p (f d) -> p f d", f=2)
                s1 = sp.tile([128, D], f32)
                nc.vector.tensor_add(
                    out=s1[:].rearrange("p (f d) -> p f d", f=1),
                    in0=s23[:, 0:1, :], in1=s23[:, 1:2, :],
                )
                # (tensor_tensor_reduce crashes the HW device; use ACT-engine
                # Square with accum_out instead, which also offloads the DVE)
                sq = sp.tile([128, F * D], f32)
                r2 = sp.tile([128, 1], f32)
                nc.scalar.activation(
                    out=sq[:], in_=hc[:], func=Act.Square, accum_out=r2[:],
                )
                ss = sp.tile([128, D], f32)
                r1 = sp.tile([128, 1], f32)
                nc.scalar.activation(
                    out=ss[:], in_=s1[:], func=Act.Square, accum_out=r1[:],
                )
                fm2 = sp.tile([128, 1], f32)
                nc.vector.tensor_sub(out=fm2[:], in0=r1[:], in1=r2[:])

                # FM linear: lin = sum_f w_lin[xoff]
                lin = sp.tile([128, 1], f32)
                nc.vector.reduce_sum(
                    out=lin[:], in_=g3[:, :, D:D + 1], axis=mybir.AxisListType.XY
                )

                # MLP in transposed-activation form
                hT_p = pp.tile([128, 128], f32)
                nc.tensor.matmul(
                    out=hT_p[:], lhsT=hc[:], rhs=blob_t[:, C_ID:C_ID + 128],
                    start=True, stop=True,
                )
                hT = sp.tile([128, 128], f32)
                nc.vector.tensor_copy(out=hT[:], in_=hT_p[:])

                a1 = sp.tile([128, H1], f32)
                for c in range(2):
                    p1 = pp.tile([128, 128], f32)
                    nc.tensor.matmul(
                        out=p1[:],
                        lhsT=blob_t[:, C_W1 + c * 128:C_W1 + (c + 1) * 128],
                        rhs=hT[:], start=True, stop=True,
                    )
                    nc.scalar.activation(
                        out=a1[:, c * 128:(c + 1) * 128], in_=p1[:], func=Act.Relu,
                        bias=blob_t[:, C_B1 + c:C_B1 + c + 1], scale=1.0,
                    )
                p2 = pp.tile([128, 128], f32)
                nc.tensor.matmul(
                    out=p2[:], lhsT=blob_t[:, C_W2:C_W2 + 128],
                    rhs=a1[:, 0:128], start=True, stop=False,
                )
                nc.tensor.matmul(
                    out=p2[:], lhsT=blob_t[:, C_W2 + 128:C_W2 + 256],
                    rhs=a1[:, 128:256], start=False, stop=True,
                )
                a2 = sp.tile([128, H2], f32)
                nc.scalar.activation(
                    out=a2[:], in_=p2[:], func=Act.Relu,
                    bias=blob_t[:, C_B2:C_B2 + 1], scale=1.0,
                )
                p3 = pp.tile([64, 128], f32)
                nc.tensor.matmul(
                    out=p3[:], lhsT=blob_t[:, C_W3:C_W3 + H3], rhs=a2[:],
                    start=True, stop=True,
                )
                a3 = sp.tile([64, 128], f32)
                nc.scalar.activation(
                    out=a3[:], in_=p3[:], func=Act.Relu,
                    bias=blob_t[0:64, C_B3:C_B3 + 1], scale=1.0,
                )
                py = pp.tile([128, 1], f32)
                nc.tensor.matmul(
                    out=py[:], lhsT=a3[:], rhs=blob_t[0:64, C_WL:C_WL + 1],
                    start=True, stop=True,
                )

                # y = 0.5*fm2 + lin + b_lin + y_dnn
                t1 = sp.tile([128, 1], f32)
                nc.vector.scalar_tensor_tensor(
                    out=t1[:], in0=fm2[:], scalar=0.5, in1=lin[:],
                    op0=Alu.mult, op1=Alu.add,
                )
                t2 = sp.tile([128, 1], f32)
                nc.vector.tensor_add(out=t2[:], in0=py[:], in1=blob_t[:, C_BLIN:C_BLIN + 1])
                nc.vector.tensor_add(out=y_sb[:, i:i + 1], in0=t1[:], in1=t2[:])

            nc.sync.dma_start(out=y[:], in_=y_sb[:])
    nc.finalize()  # runs Bacc's lowering passes; the PJRT exec path requires it
    return nc


def prepare_inputs(x, emb_table, w_lin, b_lin, w1, b1, w2, b2, w3, b3, w_last):
    x = np.asarray(x)
    xoff = (x.astype(np.int64) + OFFSETS[None, :]).astype(np.int32)  # [2048, 8]
    # per-core packed: xc[c, p, i*8+f] = xoff[c*256 + i*128 + p, f]
    xc = np.ascontiguousarray(
        xoff.reshape(N_CORES, NT, 128, F).transpose(0, 2, 1, 3).reshape(N_CORES, 128, NT * F)
    )

    tab = np.zeros((INPUT_DIM, TABW), np.float32)
    tab[:, :D] = np.asarray(emb_table, np.float32)
    tab[:, D] = np.asarray(w_lin, np.float32)

    blob = np.zeros((128, BLOBW), np.float32)
    blob[:, C_W1:C_W1 + H1] = np.asarray(w1, np.float32)
    w2 = np.asarray(w2, np.float32)
    blob[:, C_W2:C_W2 + H2] = w2[0:128, :]
    blob[:, C_W2 + H2:C_W2 + 2 * H2] = w2[128:256, :]
    blob[:, C_W3:C_W3 + H3] = np.asarray(w3, np.float32)
    blob[0:H3, C_WL] = np.asarray(w_last, np.float32)[:, 0]
    b1 = np.asarray(b1, np.float32)
    blob[:, C_B1] = b1[0:128]
    blob[:, C_B1 + 1] = b1[128:256]
    blob[:, C_B2] = np.asarray(b2, np.float32)
    blob[0:H3, C_B3] = np.asarray(b3, np.float32)
    blob[:, C_BLIN] = np.float32(np.asarray(b_lin))
    blob[:, C_ID:C_ID + 128] = np.eye(128, dtype=np.float32)
    return tab, blob, xc


def kernel(**inputs):
    tab, blob, xc = prepare_inputs(**inputs)
    if "nc" not in _CACHE:
        _CACHE["nc"] = build_program()
    nc = _CACHE["nc"]
    in_maps = [{"tab": tab, "blob": blob, "xoff": xc[c]} for c in range(N_CORES)]
    res = run_bass_kernel_spmd(nc, in_maps, list(range(N_CORES))).results
    # y[c*256 + i*128 + p] = res[c]["y"][p, i]
    out = np.concatenate([res[c]["y"].T.reshape(BC) for c in range(N_CORES)])
    return out.astype(np.float32)


if __name__ == "__main__":
    rng = np.random.default_rng(0)
    demo = {
        "x": np.stack([rng.integers(0, FIELD_DIMS[f], 2048) for f in range(F)], 1).astype(np.int64),
        "emb_table": rng.standard_normal((INPUT_DIM, D), np.float32) * 0.01,
        "w_lin": rng.random(INPUT_DIM, np.float32),
        "b_lin": np.float32(0.0),
        "w1": rng.standard_normal((F * D, H1), np.float32) * 0.1,
        "b1": np.zeros(H1, np.float32),
        "w2": rng.standard_normal((H1, H2), np.float32) * 0.1,
        "b2": np.zeros(H2, np.float32),
        "w3": rng.standard_normal((H2, H3), np.float32) * 0.1,
        "b3": np.zeros(H3, np.float32),
        "w_last": rng.standard_normal((H3, 1), np.float32) * 0.1,
    }
    print(kernel(**demo)[:8])

